# revision 4
# baseline (speedup 1.0000x reference)
"""Trainium2 Bass kernel v6 for BertWithAdaThresholdLocContextPooling.

vs v5:
- all PE transposes in bf16 (fp32 PE ops run at half rate); tanh outputs bf16
- ht normalized*256 on the single-partition row (per-partition scalar works
  there), htc fp8; rs computed as columns via DoubleRow (no rs transposes,
  no broadcast machinery); rsc cast scales by 1/256 (immediate)
- separate PSUM pairs for the two extractors (no tanh-h serialization)
- gathers at0/at1 issued before sg (they feed the longer chain)
- bulk in two issue blocks around exp/Ln on the scalar queue, first block
  gated on at1 completion; 8 pieces (no semaphore-reuse stalls)
"""

import sys

for _p in ("/opt/trn_rl_repo",):
    if _p not in sys.path:
        sys.path.insert(0, _p)

import numpy as np
import ml_dtypes

import concourse.bacc as bacc
import concourse.bass as bass
import concourse.mybir as mybir
from concourse.tile import TileContext
from concourse.tile_rust import add_dep_helper
from concourse.bass_utils import run_bass_kernel_spmd

F32 = mybir.dt.float32
BF16 = mybir.dt.bfloat16
FP8 = mybir.dt.float8e4
I32 = mybir.dt.int32
AF = mybir.ActivationFunctionType
ALU = mybir.AluOpType
DR = mybir.MatmulPerfMode.DoubleRow
NPF8 = ml_dtypes.float8_e4m3

B, L, HID = 16, 512, 768
HEADS, M = 12, 4
EMB, BLK, NER, NCLS = 768, 8, 6, 97
NCORES = 8
BPC = B // NCORES          # 2
CAT = 2 * HID + NER        # 1542
KCH = 12
NEMB = EMB // 128          # 6
NL = L // 128              # 4
NBL = EMB * BLK // 128     # 48
HTS = 256.0                # fp8 scaling for normalized ht

# csma bf16 [96, 21]: selE [0:16,0:4] | w12 [0:12,4:5] | nerb [0:6,5:9]
#   | idf4b [0:4,9:13] | fp8: selA [0:96, bf16 13:25] -> cols 13..24
SELE0, W120, NERB0, IDF40, SELA0 = 0, 4, 5, 9, 13
CSMA_COLS = 25
# csmc bf16 [6, 3286]: whner | wtner | bh | bt | selbh | selbt
#   | f32: bbrow [0:1,3080:3274] | ones2 [0:1,3274:3278]
WHN0, WTN0, BH0, BT0, SBH0, SBT0 = 0, 768, 1536, 2304, 3072, 3076
BB0, ONE0 = 3080, 3274
CSMC_COLS = 3278

_cache = {}


def _build_constants():
    csma = np.zeros((96, CSMA_COLS), ml_dtypes.bfloat16)
    for k in range(4 * M):
        csma[k, SELE0 + k // M] = 1.0
    csma[0:12, W120] = 1.0 / HEADS
    csma[0:4, IDF40:IDF40 + 4] = np.eye(4)
    av = csma.view(np.uint8)
    selA = np.zeros((96, 24), NPF8)
    for i in range(2):
        for m in range(M):
            for h in range(HEADS):
                selA[i * M * HEADS + m * HEADS + h, i * HEADS + h] = 1.0 / M
    av[:, 2 * SELA0:2 * SELA0 + 24] = selA.view(np.uint8)

    rys = np.zeros((128, BLK * 128), ml_dtypes.bfloat16)
    for y in range(BLK):
        for p in range(128):
            rys[(p // BLK) * BLK + y, y * 128 + p] = 1.0

    perm = np.empty(EMB * BLK, np.int64)
    for cch in range(NEMB):
        for y in range(BLK):
            for p in range(128):
                g = cch * 16 + p // BLK
                x = p % BLK
                perm[(cch * BLK + y) * 128 + p] = g * 64 + x * BLK + y
    return {"csma": csma, "rys": rys, "perm": perm}


def _build_program():
    nc = bacc.Bacc("TRN2", target_bir_lowering=False, debug=False)

    seq_h = nc.dram_tensor("seq", [BPC * L, HID], BF16, kind="ExternalInput")
    attn_h = nc.dram_tensor("attn", [BPC * HEADS * L, L], FP8, kind="ExternalInput")
    seqp_h = nc.dram_tensor("seqp", [128, BPC * NL * HID], FP8, kind="ExternalInput")
    cidx_h = nc.dram_tensor("cidx", [96, 3], I32, kind="ExternalInput")
    whp_hs = [nc.dram_tensor(f"whp{p}", [128, 6 * EMB], BF16, kind="ExternalInput")
              for p in range(2)]
    wtp_hs = [nc.dram_tensor(f"wtp{p}", [128, 6 * EMB], BF16, kind="ExternalInput")
              for p in range(2)]
    wbp_hs = [nc.dram_tensor(f"wbp{p}", [128, 24 * NCLS], BF16, kind="ExternalInput")
              for p in range(2)]
    csma_h = nc.dram_tensor("csma", [96, CSMA_COLS], BF16, kind="ExternalInput")
    csmc_h = nc.dram_tensor("csmc", [NER, CSMC_COLS], BF16, kind="ExternalInput")
    rys_h = nc.dram_tensor("rys", [128, BLK * 128], BF16, kind="ExternalInput")
    out_h = nc.dram_tensor("logitsT", [BPC, NCLS], F32, kind="ExternalOutput")

    with TileContext(nc) as tc:
        with (
            tc.tile_pool(name="const", bufs=1) as cp,
            tc.tile_pool(name="data", bufs=1) as dp,
            tc.tile_pool(name="psxh", bufs=1, space="PSUM") as psh,
            tc.tile_pool(name="psxt", bufs=1, space="PSUM") as pst2,
            tc.tile_pool(name="psea", bufs=1, space="PSUM") as pse,
            tc.tile_pool(name="pssm", bufs=2, space="PSUM") as pss,
            tc.tile_pool(name="pstr", bufs=1, space="PSUM") as pst,
        ):
            # ---- gpsimd: cidx then gathers (at first: they feed the chain) ----
            cidx = dp.tile([96, 3], I32)
            nc.gpsimd.dma_start(cidx[:], cidx_h[:])
            at = []
            for b in range(BPC):
                t = dp.tile([2 * M * HEADS, L], FP8, tag=f"at{b}")
                g = nc.gpsimd.indirect_dma_start(
                    out=t[:], out_offset=None, in_=attn_h[:],
                    in_offset=bass.IndirectOffsetOnAxis(ap=cidx[0:96, b:b + 1], axis=0))
                at.append(t)
            at1_gather = g
            sg = dp.tile([4 * M, HID], BF16)
            nc.gpsimd.indirect_dma_start(
                out=sg[:], out_offset=None, in_=seq_h[:],
                in_offset=bass.IndirectOffsetOnAxis(ap=cidx[0:16, 2:3], axis=0))

            # ---- sync: tiny consts ----
            csma = cp.tile([96, CSMA_COLS], BF16)
            nc.sync.dma_start(csma[:], csma_h[:])
            csmc = cp.tile([NER, CSMC_COLS], BF16)
            nc.sync.dma_start(csmc[:], csmc_h[:])

            selE = csma[0:16, SELE0:SELE0 + 4]
            w12 = csma[0:12, W120:W120 + 1]
            nerb = csma[0:NER, NERB0:NERB0 + 4]
            idfb = csma[0:4, IDF40:IDF40 + 4]
            selA = csma[0:96, SELA0:SELA0 + 12].bitcast(FP8)
            whner = csmc[0:NER, WHN0:WHN0 + EMB]
            wtner = csmc[0:NER, WTN0:WTN0 + EMB]
            bhr = csmc[0:1, BH0:BH0 + EMB]
            btr = csmc[0:1, BT0:BT0 + EMB]
            selbh = csmc[0:1, SBH0:SBH0 + 4]
            selbt = csmc[0:1, SBT0:SBT0 + 4]
            bbrow = csmc[0:1, BB0:BB0 + NCLS]
            ones2 = csmc[0:1, ONE0:ONE0 + 2]

            # ---- scalar queue: early bulk issues (gated on gathers) ----
            bulk = []
            seqp = cp.tile([128, BPC * NL * HID], FP8)
            bulk.append(nc.scalar.dma_start(seqp[:, 0:NL * HID],
                                            seqp_h[:, 0:NL * HID]))
            bulk.append(nc.scalar.dma_start(seqp[:, NL * HID:],
                                            seqp_h[:, NL * HID:]))
            whp = []
            for p in range(2):
                t = cp.tile([128, 6 * EMB], BF16, tag=f"whp{p}")
                bulk.append(nc.scalar.dma_start(t[:], whp_hs[p][:]))
                whp.append(t)
            wtpa = cp.tile([128, 6 * EMB], BF16, tag="wtp0")
            bulk.append(nc.scalar.dma_start(wtpa[:], wtp_hs[0][:]))

            # ---- spine: att pooling (PSUM-serial) + ht chains ----
            eah, ps_eat = [], []
            for b in range(BPC):
                pa = pse.tile([HEADS, L], F32, tag="ea")
                nc.tensor.matmul(pa[:], lhsT=selA[:, 0:HEADS], rhs=at[b][:],
                                 start=True, stop=True)
                e = dp.tile([HEADS, L], F32, tag=f"eah{b}")
                nc.vector.tensor_copy(e[:], pa[:])
                eah.append(e)
                pb = pse.tile([HEADS, L], F32, tag="ea")
                nc.tensor.matmul(pb[:], lhsT=selA[:, HEADS:2 * HEADS], rhs=at[b][:],
                                 start=True, stop=True)
                ps_eat.append(pb)

            # scalar: exp between the two bulk-issue blocks (Ln emitted later)
            exps = dp.tile([4 * M, HID], BF16)
            nc.scalar.activation(exps[:], sg[:], AF.Exp)

            # late bulk issues
            wtpb = cp.tile([128, 6 * EMB], BF16, tag="wtp1")
            bulk.append(nc.scalar.dma_start(wtpb[:], wtp_hs[1][:]))
            wtp = [wtpa, wtpb]
            rys = cp.tile([128, BLK * 128], BF16)
            bulk.append(nc.scalar.dma_start(rys[:], rys_h[:]))
            wbp = []
            for p in range(2):
                t = cp.tile([128, 24 * NCLS], BF16, tag=f"wbp{p}")
                bulk.append(nc.scalar.dma_start(t[:], wbp_hs[p][:]))
                wbp.append(t)
            for d in bulk:
                add_dep_helper(d.ins, at1_gather.ins,
                               reason="bulk yields HBM to gathers")

            def wchunk(pieces, j):
                return pieces[j // 6][:, (j % 6) * EMB:(j % 6 + 1) * EMB]

            # ---- per-batch ht chain -> normalized*HTS fp8 columns ----
            htc = []
            smden = dp.tile([1, 2], F32)
            for b in range(BPC):
                prd = dp.tile([HEADS, L], BF16, tag=f"prd{b}")
                nc.vector.tensor_tensor(out=prd[:], in0=eah[b][:], in1=ps_eat[b][:],
                                        op=ALU.mult)
                ps_ht = pss.tile([1, L], F32, tag="sm")
                nc.tensor.matmul(ps_ht[:], lhsT=w12, rhs=prd[:],
                                 start=True, stop=True)
                nc.vector.reduce_sum(smden[0:1, b:b + 1], ps_ht[:],
                                     axis=mybir.AxisListType.X)
                den = dp.tile([1, 1], F32, tag=f"den{b}")
                nc.vector.tensor_scalar_add(den[:], smden[0:1, b:b + 1], 1e-5)
                rcp = dp.tile([1, 1], F32, tag=f"rcp{b}")
                nc.vector.reciprocal(rcp[:], den[:])
                hr = dp.tile([1, L], BF16, tag=f"htrow{b}")
                nc.vector.tensor_scalar(hr[:], ps_ht[:], rcp[:, :1], HTS,
                                        op0=ALU.mult, op1=ALU.mult)
                ps_htc = pss.tile([128, 2 * NL], BF16, tag="sm")
                for c in range(NL):
                    nc.tensor.transpose(ps_htc[:, 2 * c:2 * c + 1],
                                        hr[:, c * 128:(c + 1) * 128],
                                        idfb[0:1, 0:1])
                h = dp.tile([128, NL], FP8, tag=f"htc{b}")
                nc.vector.tensor_copy(h[:], ps_htc[:, 0:2 * NL:2])
                htc.append(h)

            # ---- rs columns via DoubleRow (lhsT = seq chunk pairs) ----
            ps_rsccol = pst.tile([128, NEMB * BPC], F32, tag="tr")
            seqv = seqp[:].rearrange("p (b c d) -> p b c d", b=BPC, c=NL)
            for b in range(BPC):
                rv = htc[b][:].rearrange("p (i t) -> p i t", i=2)
                for d in range(NEMB):
                    for i in range(2):
                        nc.tensor.matmul(
                            ps_rsccol[:, d * BPC + b:d * BPC + b + 1],
                            lhsT=seqv[:, b, 2 * i:2 * i + 2, d * 128:(d + 1) * 128],
                            rhs=rv[:, i, :].unsqueeze(2),
                            start=(i == 0), stop=(i == 1), perf_mode=DR)
            rsc = dp.tile([128, 4 * NEMB], BF16)
            nc.vector.tensor_scalar_mul(
                rsc[:].rearrange("p (r b m) -> p r b m", r=NEMB, b=BPC),
                ps_rsccol[:].rearrange("p (r b) -> p r b", r=NEMB)
                .unsqueeze(3).broadcast_to([128, NEMB, BPC, 2]),
                1.0 / HTS)

            # entity pooling late in the PE stream (extractor is bulk-paced)
            ps_pool = pst.tile([128, 4 * NEMB], F32, tag="tr")
            for c in range(NEMB):
                nc.tensor.matmul(ps_pool[:, c * 4:(c + 1) * 4],
                                 lhsT=exps[:, c * 128:(c + 1) * 128], rhs=selE,
                                 start=True, stop=True)
            entT = dp.tile([128, 4 * NEMB], BF16)
            nc.scalar.activation(entT[:], ps_pool[:], AF.Ln)

            # ---- extractor h (own PSUM pair) ----
            ps_h = psh.tile([4, EMB], F32, tag="exh")
            for j in range(6):
                for n0, nl_ in ((0, 512), (512, 256)):
                    nc.tensor.matmul(ps_h[:, n0:n0 + nl_],
                                     lhsT=entT[:, j * 4:(j + 1) * 4],
                                     rhs=wchunk(whp, j)[:, n0:n0 + nl_],
                                     start=(j == 0), stop=False)
            for j in range(6, 12):
                for n0, nl_ in ((0, 512), (512, 256)):
                    nc.tensor.matmul(ps_h[:, n0:n0 + nl_],
                                     lhsT=rsc[:, (j - 6) * 4:(j - 5) * 4],
                                     rhs=wchunk(whp, j)[:, n0:n0 + nl_],
                                     start=False, stop=False)
            for n0, nl_ in ((0, 512), (512, 256)):
                nc.tensor.matmul(ps_h[:, n0:n0 + nl_], lhsT=nerb,
                                 rhs=whner[:, n0:n0 + nl_], start=False, stop=False)
                nc.tensor.matmul(ps_h[:, n0:n0 + nl_], lhsT=selbh,
                                 rhs=bhr[:, n0:n0 + nl_], start=False, stop=True)
            t4h = dp.tile([4, EMB], BF16, tag="t4h")
            nc.scalar.activation(t4h[:], ps_h[:], AF.Tanh)

            # ---- extractor t (own PSUM pair, starts as wtp arrives) ----
            ps_t = pst2.tile([4, EMB], F32, tag="ext")
            for j in range(12):
                lhsT = (entT[:, j * 4:(j + 1) * 4] if j < 6
                        else rsc[:, (j - 6) * 4:(j - 5) * 4])
                for n0, nl_ in ((0, 512), (512, 256)):
                    nc.tensor.matmul(ps_t[:, n0:n0 + nl_], lhsT=lhsT,
                                     rhs=wchunk(wtp, j)[:, n0:n0 + nl_],
                                     start=(j == 0), stop=False)
            for n0, nl_ in ((0, 512), (512, 256)):
                nc.tensor.matmul(ps_t[:, n0:n0 + nl_], lhsT=nerb,
                                 rhs=wtner[:, n0:n0 + nl_], start=False, stop=False)
                nc.tensor.matmul(ps_t[:, n0:n0 + nl_], lhsT=selbt,
                                 rhs=btr[:, n0:n0 + nl_], start=False, stop=True)

            # h transposes (bf16) while extractor-t runs
            ps_a = pst.tile([128, 4 * NEMB], BF16, tag="tr")
            for c in range(NEMB):
                nc.tensor.transpose(ps_a[:, c * 4:(c + 1) * 4],
                                    t4h[:, c * 128:(c + 1) * 128], idfb)
            ab = dp.tile([128, 4 * NEMB], BF16, tag="ab")
            nc.vector.tensor_copy(ab[:], ps_a[:])

            t4t = dp.tile([4, EMB], BF16, tag="t4t")
            ps_b2 = pst.tile([128, 4 * NEMB], BF16, tag="tr")
            for c in range(NEMB):
                nc.scalar.activation(t4t[:, c * 128:(c + 1) * 128],
                                     ps_t[:, c * 128:(c + 1) * 128], AF.Tanh)
                nc.tensor.transpose(ps_b2[:, c * 4:(c + 1) * 4],
                                    t4t[:, c * 128:(c + 1) * 128], idfb)
            bb2 = dp.tile([128, 4 * NEMB], BF16, tag="bb2")
            nc.vector.tensor_copy(bb2[:], ps_b2[:])

            # ---- grouped bilinear + classifier ----
            ps_t2x = pss.tile([128, BLK * NEMB * BPC], F32, tag="sm")
            tscols = bb2[:].rearrange("p (c b) -> p c b", c=NEMB)[:, :, 1:4:2]
            for y in range(BLK):
                nc.tensor.matmul(
                    ps_t2x[:, y * 12:(y + 1) * 12]
                    .rearrange("p (c b) -> p c b", c=NEMB),
                    lhsT=rys[:, y * 128:(y + 1) * 128],
                    rhs=tscols, start=True, stop=True)
            blt = dp.tile([128, NEMB * 16], BF16)
            ps_l = pss.tile([BPC, NCLS], F32, tag="sm")
            for c in range(NEMB):
                nc.vector.tensor_tensor(
                    out=blt[:, c * 16:(c + 1) * 16]
                    .rearrange("p (y b) -> p y b", y=BLK),
                    in0=ab[:, c * 4:c * 4 + 4:2].unsqueeze(1)
                        .broadcast_to([128, BLK, 2]),
                    in1=ps_t2x[:].rearrange("p (y c b) -> p y c b", y=BLK, c=NEMB)
                    [:, :, c, :],
                    op=ALU.mult)
                for y in range(BLK):
                    k = c * BLK + y
                    nc.tensor.matmul(
                        ps_l[:],
                        lhsT=blt[:, c * 16 + y * 2:c * 16 + y * 2 + 2],
                        rhs=wbp[k // 24][:, (k % 24) * NCLS:(k % 24 + 1) * NCLS],
                        start=(k == 0), stop=False)
            nc.tensor.matmul(ps_l[:], lhsT=ones2, rhs=bbrow,
                             start=False, stop=True)
            lg = dp.tile([BPC, NCLS], F32)
            nc.vector.tensor_copy(lg[:], ps_l[:])
            nc.sync.dma_start(out_h[:], lg[:])

    nc.finalize()
    return nc


def _get_program():
    if "nc" not in _cache:
        _cache["nc"] = _build_program()
        _cache["consts"] = _build_constants()
    return _cache["nc"], _cache["consts"]


def kernel(sequence_output, attention, entity_pos, hs_ner_tags, ts_ner_tags,
           Wh, bh, Wt, bt, Wb, bb):
    nc, c = _get_program()

    seq = np.asarray(sequence_output, dtype=np.float32).astype(ml_dtypes.bfloat16)
    attn = np.asarray(attention, dtype=np.float32).astype(NPF8)
    pos = np.asarray(entity_pos).astype(np.int64)
    nh = np.asarray(hs_ner_tags, dtype=np.float32)
    nt = np.asarray(ts_ner_tags, dtype=np.float32)

    whT = np.ascontiguousarray(np.asarray(Wh, dtype=np.float32).T).astype(ml_dtypes.bfloat16)
    wtT = np.ascontiguousarray(np.asarray(Wt, dtype=np.float32).T).astype(ml_dtypes.bfloat16)
    wbT = np.ascontiguousarray(np.asarray(Wb, dtype=np.float32).T)[c["perm"]]
    wbT = wbT.astype(ml_dtypes.bfloat16)

    def wimage(wT):
        img = wT[0:KCH * 128].reshape(KCH, 128, EMB).transpose(1, 0, 2)
        return np.ascontiguousarray(img.reshape(128, KCH * EMB))

    whs = wimage(whT)
    wts = wimage(wtT)
    wbs = wbT.reshape(NBL, 128, NCLS).transpose(1, 0, 2).reshape(128, NBL * NCLS)
    wbs = np.ascontiguousarray(wbs)

    csmc = np.zeros((NER, CSMC_COLS), ml_dtypes.bfloat16)
    csmc[0:NER, WHN0:WHN0 + EMB] = whT[KCH * 128:CAT]
    csmc[0:NER, WTN0:WTN0 + EMB] = wtT[KCH * 128:CAT]
    csmc[0:1, BH0:BH0 + EMB] = np.asarray(bh, np.float32).reshape(1, EMB)
    csmc[0:1, BT0:BT0 + EMB] = np.asarray(bt, np.float32).reshape(1, EMB)
    csmc[0:1, SBH0:SBH0 + 4] = np.array([1.0, 0.0, 1.0, 0.0])
    csmc[0:1, SBT0:SBT0 + 4] = np.array([0.0, 1.0, 0.0, 1.0])
    csmc[0:1, BB0:BB0 + NCLS] = np.asarray(bb, np.float32).reshape(1, NCLS)
    csmc[0:1, ONE0:ONE0 + 2] = 1.0

    in_maps = []
    for core in range(NCORES):
        b0 = core * BPC
        pc = pos[b0:b0 + BPC]
        starts = (pc + 1).astype(np.int64)
        cidx = np.zeros((96, 3), np.int32)
        for b in range(BPC):
            r = (starts[b][:, :, None] + np.arange(HEADS)[None, None, :] * L
                 + b * HEADS * L)
            cidx[:, b] = r.reshape(2 * M * HEADS)
        cidx[0:4 * M, 2] = (starts + (np.arange(BPC) * L)[:, None, None]).reshape(4 * M)
        ner = np.stack([nh[b0], nt[b0], nh[b0 + 1], nt[b0 + 1]], axis=1)
        csma = c["csma"].copy()
        csma[0:NER, NERB0:NERB0 + 4] = ner.astype(ml_dtypes.bfloat16)

        seqb = np.ascontiguousarray(seq[b0:b0 + BPC])
        seqp = np.asarray(seqb, np.float32).astype(NPF8)
        seqp = seqp.reshape(BPC, NL, 128, HID).transpose(2, 0, 1, 3)
        seqp = np.ascontiguousarray(seqp.reshape(128, BPC * NL * HID))

        im = {
            "seq": seqb.reshape(BPC * L, HID),
            "attn": np.ascontiguousarray(attn[b0:b0 + BPC]).reshape(BPC * HEADS * L, L),
            "seqp": seqp,
            "cidx": cidx,
            "csma": csma, "csmc": csmc, "rys": c["rys"],
        }
        for p in range(2):
            im[f"whp{p}"] = np.ascontiguousarray(whs[:, p * 6 * EMB:(p + 1) * 6 * EMB])
            im[f"wtp{p}"] = np.ascontiguousarray(wts[:, p * 6 * EMB:(p + 1) * 6 * EMB])
            im[f"wbp{p}"] = np.ascontiguousarray(
                wbs[:, p * 24 * NCLS:(p + 1) * 24 * NCLS])
        in_maps.append(im)

    res = run_bass_kernel_spmd(nc, in_maps, core_ids=list(range(NCORES)))
    _cache["last_res"] = res
    out = np.empty((B, NCLS), np.float32)
    for core in range(NCORES):
        out[core * BPC:(core + 1) * BPC] = res.results[core]["logitsT"]
    return out
